# revision 2
# baseline (speedup 1.0000x reference)
"""Trainium2 Bass kernel for nn_AccuracyMetricLoss.

Computes mean over 200000 days of per-day scores:
    denom = max(t, 0.2*cap);  rel_sq = ((t-p)/denom)^2
    score_d = (1 - sqrt(mean_96(rel_sq))) * 100;  out = mean_d(score_d)

Sharding: day axis split evenly across 8 NeuronCores (25000 days/core).

Per-core pipeline, per [P, C_DAYS*96] tile (day-major, contiguous rows):
    DMA t (SP ring), p (ACT ring) -> SBUF
    DVE:  d = t - p              (in-place into p)
    ACT:  lt = ln(t); r2 = exp(-2*lt) = 1/t^2   (in-place into t; one table set)
    DVE:  custom fused op  s = cumsum(d^2 * min(r2, 1/thresh^2))  (into lt)
    GpSimd: copy strided per-day samples s[:, 95::96] into acc
    single final DMA of acc -> DRAM
Host: difference the prefix samples -> per-day sums, sqrt/score/mean in f64.
"""
import os
import sys

sys.path.insert(0, "/opt/trn_rl_repo")

import numpy as np

import concourse.bacc as bacc
import concourse.mybir as mybir
from concourse.bass_utils import run_bass_kernel_spmd
from concourse.tile import TileContext

from concourse.dve_ops import DveOp, OPS, CUSTOM_DVE_SPECS, _SUB_OPCODE_FOR_NAME
from concourse.dve_spec import Spec, Src0, Src1, C0, AluOp, sq, minn, scan, lower
from concourse.dve_uop import DveOpSpec

# ---------------- problem constants (hardcoded) ---------------- #
CAP = (300 + 400 + 900) / 300 / 1000 * 300400.0  # 1602.1333...
THRESH = np.float32(0.2) * np.float32(CAP)
C2 = float(1.0 / (np.float64(THRESH) ** 2))  # clamp for 1/t^2
T = 96
N_DAYS = 200000
N_CORES = 8
DAYS_PER_CORE = N_DAYS // N_CORES  # 25000
C_DAYS = 50  # days per partition row per tile
FD = C_DAYS * T  # 4800
P = 128
# per core: 3 full [128 x 50d] tiles + one [116 x 50d] remainder tile
TILE_ROWS = [128, 128, 128, 116]
N_TILES = len(TILE_ROWS)
assert sum(r * C_DAYS for r in TILE_ROWS) == DAYS_PER_CORE


def _register_clamp_sq_scan():
    name = "CLAMP_SQ_SCAN_ANT"
    for op in OPS:
        if op.name == name:
            return op

    body = scan(AluOp.ADD, sq(Src0) * minn(Src1, C0))

    def _ref(in0, in1, s0, s1, imm2):
        x = np.asarray(in0, np.float32)
        r = np.asarray(in1, np.float32).reshape(x.shape[0], -1)
        c = s0 if isinstance(s0, float) else np.asarray(s0, np.float32).reshape(-1, 1)
        b = (x.reshape(x.shape[0], -1) ** 2) * np.minimum(r, c)
        out = np.cumsum(b.astype(np.float32), axis=-1, dtype=np.float32)
        return out.reshape(in0.shape)

    spec = Spec(body=body, reference=_ref)
    row = 1 + len(OPS)
    assert row < 0x20
    _SUB_OPCODE_FOR_NAME[name] = row
    shas = {}
    for ver in ("v3", "v4"):
        u = lower(spec, ver=ver)
        shas[ver] = DveOpSpec(name=name, opcode=row, uops=u, rd1_en=True).sha(ver)
    op = DveOp(name, spec, subdim=False, uops_sha=shas)
    OPS.append(op)
    CUSTOM_DVE_SPECS[name] = spec
    return op


def _pin_act_table_set():
    """Make Ln and Exp resolve only to natural_log_exp_and_others so the
    table-load pass emits one hoisted load instead of alternating reloads.
    Mutates the functools.cache'd dict in place (order/len preserved)."""
    from concourse.hw_specs import get_activation_tables

    tables = get_activation_tables("gen3")
    keep = "natural_log_exp_and_others"
    if keep not in tables:
        return
    for name, fns in tables.items():
        if name == keep:
            continue
        fns.discard(mybir.ActivationFunctionType.Ln)
        fns.discard(mybir.ActivationFunctionType.Exp)


_nc_cache = {}


def _build_nc():
    if "nc" in _nc_cache:
        return _nc_cache["nc"]
    clamp_sq_scan = _register_clamp_sq_scan()
    _pin_act_table_set()

    nc = bacc.Bacc("TRN2")
    n_elem = DAYS_PER_CORE * T
    t_in = nc.dram_tensor("t_in", [n_elem], mybir.dt.float32, kind="ExternalInput")
    p_in = nc.dram_tensor("p_in", [n_elem], mybir.dt.float32, kind="ExternalInput")
    out = nc.dram_tensor(
        "out", [P, N_TILES * C_DAYS], mybir.dt.float32, kind="ExternalOutput"
    )

    with TileContext(nc) as tc:
        with (
            tc.tile_pool(name="tp", bufs=2) as tp,
            tc.tile_pool(name="pp", bufs=2) as pp,
            tc.tile_pool(name="lp", bufs=2) as lp,
            tc.tile_pool(name="accp", bufs=1) as accp,
        ):
            acc = accp.tile([P, N_TILES * C_DAYS], mybir.dt.float32)
            for i, rows in enumerate(TILE_ROWS):
                base = i * 128 * FD
                t_v = t_in[base : base + rows * FD].rearrange("(p f) -> p f", p=rows)
                p_v = p_in[base : base + rows * FD].rearrange("(p f) -> p f", p=rows)

                t = tp.tile([P, FD], mybir.dt.float32)
                p = pp.tile([P, FD], mybir.dt.float32)
                lt = lp.tile([P, FD], mybir.dt.float32)

                nc.sync.dma_start(out=t[:rows, :], in_=t_v)
                nc.scalar.dma_start(out=p[:rows, :], in_=p_v)
                # lt = ln(t)
                nc.scalar.activation(
                    lt[:rows, :], t[:rows, :], mybir.ActivationFunctionType.Ln
                )
                # d = t - p   (in place into p)
                nc.vector.tensor_tensor(
                    p[:rows, :], t[:rows, :], p[:rows, :], mybir.AluOpType.subtract
                )
                # r2 = exp(-2*lt) = 1/t^2   (in place into t)
                nc.scalar.activation(
                    t[:rows, :],
                    lt[:rows, :],
                    mybir.ActivationFunctionType.Exp,
                    scale=-2.0,
                )
                # s = cumsum(d^2 * min(r2, C2))  (into lt)
                nc.vector._custom_dve(
                    clamp_sq_scan,
                    out=lt[:rows, :],
                    in0=p[:rows, :],
                    in1=t[:rows, :],
                    s0=C2,
                )
                # collect per-day prefix samples into acc
                samples = lt[:rows, :].rearrange("p (c n) -> p c n", n=T)[:, :, 95]
                nc.gpsimd.tensor_copy(
                    acc[:rows, i * C_DAYS : (i + 1) * C_DAYS], samples
                )
            nc.sync.dma_start(out=out[:], in_=acc[:])
    nc.finalize()
    _nc_cache["nc"] = nc
    return nc


_last_results = None


def kernel(pred: np.ndarray, true: np.ndarray) -> np.ndarray:
    global _last_results
    nc = _build_nc()

    n_elem = DAYS_PER_CORE * T
    pred = np.ascontiguousarray(pred, dtype=np.float32)
    true = np.ascontiguousarray(true, dtype=np.float32)
    in_maps = [
        {
            "t_in": true[k * n_elem : (k + 1) * n_elem],
            "p_in": pred[k * n_elem : (k + 1) * n_elem],
        }
        for k in range(N_CORES)
    ]

    trace = bool(os.environ.get("BASS_TRACE"))
    res = run_bass_kernel_spmd(nc, in_maps, list(range(N_CORES)), trace=trace)
    _last_results = res

    # host-side tail: prefix samples -> day sums -> scores -> mean
    total = 0.0
    for k in range(N_CORES):
        A = res.results[k]["out"].astype(np.float64)  # [128, N_TILES*C_DAYS]
        A = A.reshape(P, N_TILES, C_DAYS)
        u = A.copy()
        u[:, :, 1:] -= A[:, :, :-1]  # per-day sums of rel_sq
        scores = (1.0 - np.sqrt(u / T)) * 100.0
        for i, rows in enumerate(TILE_ROWS):
            total += scores[:rows, i, :].sum()
    return np.float32(total / N_DAYS)
